# revision 4
# baseline (speedup 1.0000x reference)
"""TRN2 Bass kernel for nn_BlockLinear: per token t (32768 of them),
x_t [32,128] -> P(P(x_t@w1)@w2) where P(Y) = reshape(Y.T, (32,128)).

Strategy (data-parallel over 8 NeuronCores, 4096 tokens/core):
  - Host: round x/w to fp32r (12-bit significand; PE runs fp32r matmuls at
    bf16 speed), permute weight columns so the inter-stage permutation
    becomes a 32x32 blockwise transpose (DVE native op).
  - Layout: one whole token per SBUF partition at both DMA ends -> 16 KiB
    contiguous HBM runs (full DMA rate). Token structure is recovered
    on-chip with PE transposes.
  - Per 128-token chunk (2 MiB):
      load XB[tok, (b,m)] -> 32x PE-T -> Xt[m, (T,b)] -> mm1(w1p) ->
      DVE 32x32-T -> fp32r cast -> mm2(w2p) -> DVE 32x32-T (restrided) ->
      32x PE-T -> OB[tok, (i,j)] -> store.
"""
import numpy as np
from contextlib import ExitStack

import concourse.bass as bass
from concourse import bacc
import concourse.tile as tile
from concourse import mybir
from concourse.bass_utils import run_bass_kernel_spmd

F32 = mybir.dt.float32
F32R = mybir.dt.float32r

N_CORES = 8
TOK_PER_CORE = 4096
CHUNK_TOK = 128          # one token per partition
N = 4096                 # elems per token


def _round_f32r(a):
    u = np.ascontiguousarray(a).view(np.uint32)
    r = ((u.astype(np.uint64) + 0x800) & 0xFFFFF000).astype(np.uint32)
    return r.view(np.float32)


def _perm():
    p = np.zeros(128, np.int64)
    for h in range(4):
        for i in range(32):
            p[32 * h + i] = 4 * i + h
    return p


def build_nc(ntok):
    nchunks = ntok // CHUNK_TOK
    nc = bacc.Bacc("TRN2", target_bir_lowering=False, debug=False)
    X = nc.dram_tensor("x", [ntok, N], F32R, kind="ExternalInput").ap()
    W1 = nc.dram_tensor("w1p", [128, 128], F32R, kind="ExternalInput").ap()
    W2 = nc.dram_tensor("w2p", [128, 128], F32R, kind="ExternalInput").ap()
    IR = nc.dram_tensor("ident_r", [128, 128], F32R, kind="ExternalInput").ap()
    IF = nc.dram_tensor("ident_f", [128, 128], F32, kind="ExternalInput").ap()
    OUT = nc.dram_tensor("out", [ntok, N], F32, kind="ExternalOutput").ap()

    with tile.TileContext(nc) as tc, ExitStack() as ctx:
        wpool = ctx.enter_context(tc.tile_pool(name="w", bufs=1))
        xbp = ctx.enter_context(tc.tile_pool(name="xbp", bufs=3))
        zrp = ctx.enter_context(tc.tile_pool(name="zrp", bufs=2))
        xtp = ctx.enter_context(tc.tile_pool(name="xtp", bufs=2))
        ztp = ctx.enter_context(tc.tile_pool(name="ztp", bufs=2))
        h2p = ctx.enter_context(tc.tile_pool(name="h2p", bufs=2))
        obp = ctx.enter_context(tc.tile_pool(name="obp", bufs=2))
        psp = ctx.enter_context(tc.tile_pool(name="psp", bufs=2, space="PSUM"))

        w1_sb = wpool.tile([128, 128], F32R)
        w2_sb = wpool.tile([128, 128], F32R)
        ir_sb = wpool.tile([128, 128], F32R)
        if_sb = wpool.tile([128, 128], F32)
        nc.sync.dma_start(w1_sb[:], W1[:])
        nc.sync.dma_start(w2_sb[:], W2[:])
        nc.sync.dma_start(ir_sb[:], IR[:])
        nc.sync.dma_start(if_sb[:], IF[:])

        for c in range(nchunks):
            # 1. load: XB[p, (b,m)] = x[c*128 + p, b, m]; 16 KiB/partition
            xb = xbp.tile([128, N], F32R, tag="xb")
            nc.sync.dma_start(xb[:], X[c * CHUNK_TOK:(c + 1) * CHUNK_TOK, :])

            # 2./3. T_in (groups of 8) + scatter-evac at FD=1024: Xt[m, 32T+b]
            xt = xtp.tile([128, N], F32R, tag="xt")
            for b8 in range(4):
                tin = psp.tile([128, 1024], F32R, tag="tp")
                for bb in range(8):
                    b = 8 * b8 + bb
                    nc.tensor.transpose(
                        tin[:, bass.ts(bb, 128)], xb[:, bass.ts(b, 128)], ir_sb[:]
                    )
                # out positions 32T + 8*b8 + bb -> AP dims [bb(1,8), T(32,128)]
                dst = xt[:].rearrange("p (t b) -> p b t", b=32)[:, bass.ts(b8, 8), :]
                src = tin[:].rearrange("p (b t) -> p b t", b=8)
                nc.scalar.copy(dst, src)

            # 4./5. mm1 + VT1 at FD=1024 -> zraw (fp32), then 6. cast to f32r
            zt = ztp.tile([128, N], F32R, tag="zt")
            for q2 in range(4):
                y1 = psp.tile([128, 1024], F32, tag="y")
                for qq in range(2):
                    q = 2 * q2 + qq
                    nc.tensor.matmul(y1[:, bass.ts(qq, 512)], w1_sb[:],
                                     xt[:, bass.ts(q, 512)], start=True, stop=True)
                zraw = zrp.tile([128, 1024], F32, tag="zraw")
                nc.vector.transpose(zraw[:], y1[:])
                # cast split: 2x gpsimd, 1x scalar, 1x vector per chunk
                dst = zt[:, bass.ts(q2, 1024)]
                if q2 < 2:
                    nc.gpsimd.tensor_copy(dst, zraw[:])
                elif q2 == 2:
                    nc.scalar.copy(dst, zraw[:])
                else:
                    nc.vector.tensor_copy(dst, zraw[:])

            # 7./8. mm2 + VT2 at FD=1024 (restride (T,i)->(i,T)): H2[j, 128i+T]
            h2 = h2p.tile([128, N], F32, tag="h2")
            h2v = h2[:].rearrange("p (i t) -> p t i", i=32)  # [p, T(128), i(32)]
            for q2 in range(4):
                y2 = psp.tile([128, 1024], F32, tag="y")
                for qq in range(2):
                    q = 2 * q2 + qq
                    nc.tensor.matmul(y2[:, bass.ts(qq, 512)], w2_sb[:],
                                     zt[:, bass.ts(q, 512)], start=True, stop=True)
                # stream free order = (T in [32q2, 32q2+32), i in [0,32))
                nc.vector.transpose(h2v[:, bass.ts(q2, 32), :], y2[:])

            # 9./10. T_out (groups of 8) + evac2 at FD=1024: OB[T, 128i+j]
            ob = obp.tile([128, N], F32, tag="ob")
            for i8 in range(4):
                tout = psp.tile([128, 1024], F32, tag="tp")
                for ii in range(8):
                    i = 8 * i8 + ii
                    nc.tensor.transpose(
                        tout[:, bass.ts(ii, 128)], h2[:, bass.ts(i, 128)], if_sb[:]
                    )
                nc.scalar.copy(ob[:, bass.ts(i8, 1024)], tout[:])

            # 11. store
            nc.sync.dma_start(OUT[c * CHUNK_TOK:(c + 1) * CHUNK_TOK, :], ob[:])

    if not nc.is_finalized():
        nc.finalize()
    return nc


_NC_CACHE = {}


def _get_nc(ntok):
    if ntok not in _NC_CACHE:
        _NC_CACHE[ntok] = build_nc(ntok)
    return _NC_CACHE[ntok]


def kernel(x, w1, w2):
    """x [8, 4096, 4096] f32; w1, w2 [128, 128] f32 -> [8, 4096, 4096] f32."""
    lead = x.shape[:-1]
    xf = np.ascontiguousarray(x, dtype=np.float32).reshape(-1, N)
    ntok_total = xf.shape[0]
    assert ntok_total % N_CORES == 0
    ntok = ntok_total // N_CORES

    perm = _perm()
    w1p = _round_f32r(np.ascontiguousarray(w1, np.float32)[:, perm])
    w2p = _round_f32r(np.ascontiguousarray(w2, np.float32)[:, perm])
    ident = np.eye(128, dtype=np.float32)
    xr = _round_f32r(xf)

    nc = _get_nc(ntok)
    in_maps = []
    for i in range(N_CORES):
        in_maps.append({
            "x": xr[i * ntok:(i + 1) * ntok],
            "w1p": w1p, "w2p": w2p,
            "ident_r": ident, "ident_f": ident,
        })
    res = run_bass_kernel_spmd(nc, in_maps, list(range(N_CORES)))
    out = np.empty((ntok_total, N), np.float32)
    for i in range(N_CORES):
        out[i * ntok:(i + 1) * ntok] = res.results[i]["out"]
    return out.reshape(*lead, N)


# revision 5
# speedup vs baseline: 1.6132x; 1.6132x over previous
"""TRN2 Bass kernel for nn_BlockLinear: per token t (32768 of them),
x_t [32,128] -> P(P(x_t@w1)@w2) where P(Y) = reshape(Y.T, (32,128)).

Strategy (data-parallel over 8 NeuronCores, 4096 tokens/core):
  - Host: round x/w to fp32r (12-bit significand; PE runs fp32r matmuls at
    bf16 speed), permute weight columns so the inter-stage permutation
    becomes a 32x32 blockwise transpose (DVE native op).
  - Layout: one whole token per SBUF partition at both DMA ends -> 16 KiB
    contiguous HBM runs (full DMA rate). Token structure is recovered
    on-chip with PE transposes.
  - Per 128-token chunk (2 MiB):
      load XB[tok, (b,m)] -> 32x PE-T -> Xt[m, (T,b)] -> mm1(w1p) ->
      DVE 32x32-T -> fp32r cast -> mm2(w2p) -> DVE 32x32-T (restrided) ->
      32x PE-T -> OB[tok, (i,j)] -> store.
"""
import numpy as np
from contextlib import ExitStack

import concourse.bass as bass
from concourse import bacc
import concourse.tile as tile
from concourse import mybir
from concourse.bass_utils import run_bass_kernel_spmd

F32 = mybir.dt.float32
F32R = mybir.dt.float32r

N_CORES = 8
TOK_PER_CORE = 4096
CHUNK_TOK = 128          # one token per partition
N = 4096                 # elems per token


def _round_f32r(a):
    u = np.ascontiguousarray(a).view(np.uint32)
    r = ((u.astype(np.uint64) + 0x800) & 0xFFFFF000).astype(np.uint32)
    return r.view(np.float32)


def _perm():
    p = np.zeros(128, np.int64)
    for h in range(4):
        for i in range(32):
            p[32 * h + i] = 4 * i + h
    return p


def build_nc(ntok):
    nchunks = ntok // CHUNK_TOK
    nc = bacc.Bacc("TRN2", target_bir_lowering=False, debug=False)
    X = nc.dram_tensor("x", [ntok, N], F32R, kind="ExternalInput").ap()
    W1 = nc.dram_tensor("w1p", [128, 128], F32R, kind="ExternalInput").ap()
    W2 = nc.dram_tensor("w2p", [128, 128], F32R, kind="ExternalInput").ap()
    IR = nc.dram_tensor("ident_r", [128, 128], F32R, kind="ExternalInput").ap()
    IF = nc.dram_tensor("ident_f", [128, 128], F32, kind="ExternalInput").ap()
    OUT = nc.dram_tensor("out", [ntok, N], F32, kind="ExternalOutput").ap()

    with tile.TileContext(nc) as tc, ExitStack() as ctx:
        wpool = ctx.enter_context(tc.tile_pool(name="w", bufs=1))
        xbp = ctx.enter_context(tc.tile_pool(name="xbp", bufs=2))
        zrp = ctx.enter_context(tc.tile_pool(name="zrp", bufs=4))
        xtp = ctx.enter_context(tc.tile_pool(name="xtp", bufs=2))
        ztp = ctx.enter_context(tc.tile_pool(name="ztp", bufs=2))
        gp_ = ctx.enter_context(tc.tile_pool(name="gp", bufs=2))
        obp = ctx.enter_context(tc.tile_pool(name="obp", bufs=2))
        # PSUM: tag "a" shared {T_in, mm1}, tag "b" shared {mm2, T_out}.
        # Both pairings follow chunk program order (no cross-chunk stalls).
        psp = ctx.enter_context(tc.tile_pool(name="psp", bufs=2, space="PSUM"))

        w1_sb = wpool.tile([128, 128], F32R)
        w2_sb = wpool.tile([128, 128], F32R)
        ir_sb = wpool.tile([128, 128], F32R)
        if_sb = wpool.tile([128, 128], F32)
        nc.sync.dma_start(w1_sb[:], W1[:])
        nc.sync.dma_start(w2_sb[:], W2[:])
        nc.sync.dma_start(ir_sb[:], IR[:])
        nc.sync.dma_start(if_sb[:], IF[:])

        for c in range(nchunks):
            # 1. load: XB[p, (b,m)] = x[c*128 + p, b, m]; 16 KiB/partition
            xb = xbp.tile([128, N], F32R, tag="xb")
            nc.sync.dma_start(xb[:], X[c * CHUNK_TOK:(c + 1) * CHUNK_TOK, :])

            # 2./3. T_in (groups of 8) + scatter-evac at FD=1024: Xt[m, 32T+b]
            xt = xtp.tile([128, N], F32R, tag="xt")
            for b8 in range(4):
                tin = psp.tile([128, 1024], F32R, tag="a")
                for bb in range(8):
                    b = 8 * b8 + bb
                    nc.tensor.transpose(
                        tin[:, bass.ts(bb, 128)], xb[:, bass.ts(b, 128)], ir_sb[:]
                    )
                # out positions 32T + 8*b8 + bb -> AP dims [bb(1,8), T(32,128)]
                dst = xt[:].rearrange("p (t b) -> p b t", b=32)[:, bass.ts(b8, 8), :]
                src = tin[:].rearrange("p (b t) -> p b t", b=8)
                nc.scalar.copy(dst, src)

            # 4./5. mm1 + VT1 at FD=1024 -> zraw (fp32), then 6. cast to f32r
            zt = ztp.tile([128, N], F32R, tag="zt")
            for q2 in range(4):
                y1 = psp.tile([128, 1024], F32, tag="a")
                for qq in range(2):
                    q = 2 * q2 + qq
                    nc.tensor.matmul(y1[:, bass.ts(qq, 512)], w1_sb[:],
                                     xt[:, bass.ts(q, 512)], start=True, stop=True)
                zraw = zrp.tile([128, 1024], F32, tag="zraw")
                nc.vector.transpose(zraw[:], y1[:])
                # cast split: gpsimd x2, scalar x1, vector x1 per chunk
                dst = zt[:, bass.ts(q2, 1024)]
                if q2 < 2:
                    nc.gpsimd.tensor_copy(dst, zraw[:])
                elif q2 == 2:
                    nc.scalar.copy(dst, zraw[:])
                else:
                    nc.vector.tensor_copy(dst, zraw[:])

            # 7./8. mm2 + VT2 at FD=1024 (contiguous out): G[j, 32T + i]
            g = gp_.tile([128, N], F32, tag="g")
            for q2 in range(4):
                y2 = psp.tile([128, 1024], F32, tag="b")
                for qq in range(2):
                    q = 2 * q2 + qq
                    nc.tensor.matmul(y2[:, bass.ts(qq, 512)], w2_sb[:],
                                     zt[:, bass.ts(q, 512)], start=True, stop=True)
                nc.vector.transpose(g[:, bass.ts(q2, 1024)], y2[:])

            # 9./10. T_out (strided lhsT: U_i = G[:, i::32]) + evac2 FD=1024
            ob = obp.tile([128, N], F32, tag="ob")
            gv = g[:].rearrange("p (t i) -> p i t", i=32)  # [p, i(32), T(128)]
            for i8 in range(4):
                tout = psp.tile([128, 1024], F32, tag="b")
                for ii in range(8):
                    i = 8 * i8 + ii
                    nc.tensor.transpose(
                        tout[:, bass.ts(ii, 128)], gv[:, i, :], if_sb[:]
                    )
                nc.scalar.copy(ob[:, bass.ts(i8, 1024)], tout[:])

            # 11. store
            nc.sync.dma_start(OUT[c * CHUNK_TOK:(c + 1) * CHUNK_TOK, :], ob[:])

    if not nc.is_finalized():
        nc.finalize()
    return nc


_NC_CACHE = {}


def _get_nc(ntok):
    if ntok not in _NC_CACHE:
        _NC_CACHE[ntok] = build_nc(ntok)
    return _NC_CACHE[ntok]


def kernel(x, w1, w2):
    """x [8, 4096, 4096] f32; w1, w2 [128, 128] f32 -> [8, 4096, 4096] f32."""
    lead = x.shape[:-1]
    xf = np.ascontiguousarray(x, dtype=np.float32).reshape(-1, N)
    ntok_total = xf.shape[0]
    assert ntok_total % N_CORES == 0
    ntok = ntok_total // N_CORES

    perm = _perm()
    w1p = _round_f32r(np.ascontiguousarray(w1, np.float32)[:, perm])
    w2p = _round_f32r(np.ascontiguousarray(w2, np.float32)[:, perm])
    ident = np.eye(128, dtype=np.float32)
    xr = _round_f32r(xf)

    nc = _get_nc(ntok)
    in_maps = []
    for i in range(N_CORES):
        in_maps.append({
            "x": xr[i * ntok:(i + 1) * ntok],
            "w1p": w1p, "w2p": w2p,
            "ident_r": ident, "ident_f": ident,
        })
    res = run_bass_kernel_spmd(nc, in_maps, list(range(N_CORES)))
    out = np.empty((ntok_total, N), np.float32)
    for i in range(N_CORES):
        out[i * ntok:(i + 1) * ntok] = res.results[i]["out"]
    return out.reshape(*lead, N)
